# revision 14
# baseline (speedup 1.0000x reference)
"""DNAMite (dense_mlp) Trainium2 kernel.

Strategy
--------
The model is, per batch row b:
  out[b] = sum_p z_p * MLP_p(concat(emb[x_bp], emb[y_bp]))      (2016 pair MLPs)
         + sum_f z_f * MLP_f(emb[m_bf])                         (64 main MLPs)

Device-side work per (pair|feature) task, with batch B=512 as the matmul
moving dimension:
  1. h0   = W0stack.T @ onehot              (K=128, M=128, N=512, bf16)
  2. a0   = relu(h0)                        (DVE tensor_scalar, PSUM bf16 -> SBUF bf16)
  3. h1   = W1.T @ a0                       (K=128, M=128, N=512, bf16)
  4. a1   = relu(h1 + b1)                   (alternating DVE / ACT)
  5. outP += w2z.T @ a1                     (K=128, M=1, N=512 -> single PSUM fp32 bank,
                                             accumulated across ALL tasks)

Host-side folding (sample-independent weight transforms):
  * The embedding gather + layer-0 matmul are fused:
      W0stack[p] = [emb_i @ pw0[:,:E,:] + pb0 ; emb_j @ pw0[:,E:,:]]  (128x128)
    and the gather itself becomes a {0,1} one-hot rhs built from the integer
    bin indices (exact in bf16).  pb0 folds into the x-half rows since every
    one-hot column has exactly one hit there.
  * z gates fold into the layer-2 weights; z.b2 terms are a scalar constant.

Sharding: 2016 pairs -> 252/core, 64 main features -> 8/core (8 cores).
Each core returns a [1, 512] fp32 partial; the host sums them + constant.
"""

import sys

import numpy as np

for _p in ("/opt/trn_rl_repo",):
    if _p not in sys.path:
        sys.path.insert(0, _p)

import ml_dtypes

import concourse.bass as bass
import concourse.mybir as mybir
import concourse.tile as tile
from concourse.bass_utils import run_bass_kernel_spmd

BF16 = ml_dtypes.bfloat16

N_CORES = 8
B = 512
NF = 64
BINS = 64
E = 32
H = 128
P_TOT = NF * (NF - 1) // 2  # 2016
PPC = P_TOT // N_CORES      # 252 pair tasks per core
FPC = NF // N_CORES         # 8 main-feature tasks per core
T = PPC + FPC               # 260 tasks per core
CH = 10                     # tasks per DMA chunk
assert T % CH == 0
PREFETCH = 3                # chunks of DMA lookahead

# free-dim layout of one task row in the mega stream:
#   [0:128)   W0stack lhsT   [K=128 stacked-bin rows, M=128 hidden]
#   [128:256) W1 lhsT        [K=128 hidden, M=128 hidden]
#   [256:768) onehot rhs     [128 stacked-bin rows, B=512]
MEGA_F = 128 + 128 + B

_cache: dict = {}


def _build_bass():
    nc = bass.Bass()
    mega = nc.dram_tensor("mega", [128, T, MEGA_F], mybir.dt.bfloat16,
                          kind="ExternalInput")
    pbt = nc.dram_tensor("pbt", [128, T], mybir.dt.float32, kind="ExternalInput")
    w2t = nc.dram_tensor("w2t", [128, T], mybir.dt.bfloat16, kind="ExternalInput")
    out = nc.dram_tensor("out", [1, B], mybir.dt.float32, kind="ExternalOutput")

    with tile.TileContext(nc) as tc:
        with (
            tc.tile_pool(name="const", bufs=1) as cpool,
            tc.tile_pool(name="mega", bufs=5) as mpool,
            tc.tile_pool(name="act", bufs=12) as apool,
            tc.tile_pool(name="ps", bufs=3, space="PSUM") as pspool,
            tc.tile_pool(name="po", bufs=1, space="PSUM") as popool,
        ):
            pbt_sb = cpool.tile([128, T], mybir.dt.float32)
            w2t_sb = cpool.tile([128, T], mybir.dt.bfloat16)
            nc.sync.dma_start(pbt_sb[:], pbt[:])
            nc.sync.dma_start(w2t_sb[:], w2t[:])

            # warmup read of pbt_sb on ACT so steady-state activations don't
            # need a DMA wait (walrus allows a single sync wait on ACT ops)
            warm = cpool.tile([128, 1], mybir.dt.float32)
            nc.scalar.activation(warm[:], pbt_sb[:, 0:1],
                                 mybir.ActivationFunctionType.Relu,
                                 bias=0.0, scale=1.0)

            out_ps = popool.tile([1, B], mybir.dt.float32)

            # Software-pipelined emission: per pseudo-iteration u, the PE
            # stream is mm1(u), mm2(u-2), mm3(u-4) — every matmul's input was
            # produced >=2 iterations earlier, so PE never head-of-line stalls
            # on a DVE/ACT result.
            mega_tiles = {}   # chunk -> tile
            h0_t, a0_t, h1_t, a1_t = {}, {}, {}, {}

            def fetch_chunk(c):
                if c < 0 or c >= T // CH or c in mega_tiles:
                    return
                mt = mpool.tile([128, CH, MEGA_F], mybir.dt.bfloat16)
                nc.sync.dma_start(mt[:], mega[:, c * CH:(c + 1) * CH, :])
                mega_tiles[c] = mt

            for c in range(PREFETCH):
                fetch_chunk(c)

            for u in range(T + 4):
                if u < T:
                    t = u
                    if t % CH == 0:
                        fetch_chunk(t // CH + PREFETCH)
                    mt = mega_tiles[t // CH]
                    h0 = pspool.tile([128, B], mybir.dt.float32, tag="h0")
                    nc.tensor.matmul(h0[:], mt[:, t % CH, 0:128],
                                     mt[:, t % CH, 256:256 + B],
                                     start=True, stop=True)
                    h0_t[t] = h0
                if u >= 1 and u - 1 < T:
                    t = u - 1
                    a0 = apool.tile([128, B], mybir.dt.bfloat16, tag="a0")
                    nc.vector.tensor_scalar(a0[:], h0_t.pop(t)[:], 0.0, None,
                                            mybir.AluOpType.max)
                    a0_t[t] = a0
                if u >= 2 and u - 2 < T:
                    t = u - 2
                    mt = mega_tiles[t // CH]
                    h1 = pspool.tile([128, B], mybir.dt.float32, tag="h1")
                    nc.tensor.matmul(h1[:], mt[:, t % CH, 128:256],
                                     a0_t.pop(t)[:], start=True, stop=True)
                    h1_t[t] = h1
                if u >= 3 and u - 3 < T:
                    t = u - 3
                    a1 = apool.tile([128, B], mybir.dt.bfloat16, tag="a1")
                    nc.scalar.activation(a1[:], h1_t.pop(t)[:],
                                         mybir.ActivationFunctionType.Relu,
                                         bias=pbt_sb[:, t:t + 1], scale=1.0)
                    a1_t[t] = a1
                if u >= 4:
                    t = u - 4
                    nc.tensor.matmul(out_ps[:], w2t_sb[:, t:t + 1],
                                     a1_t.pop(t)[:],
                                     start=(t == 0), stop=(t == T - 1))

            out_sb = cpool.tile([1, B], mybir.dt.float32)
            nc.vector.tensor_copy(out_sb[:], out_ps[:])
            nc.sync.dma_start(out[:], out_sb[:])

    _split_excess_waits(nc)
    return nc


# Engine-compute opcodes whose ISA structs only carry ONE sync-wait slot in
# this walrus build ("Too many sync wait commands" otherwise).
_SPLIT_TYPES = (
    "InstTensorScalarPtr",
    "InstActivation",
    "InstTensorCopy",
    "InstTensorTensor",
    "InstTensorReduce",
    "InstDMACopy",
    "InstDrain",
)


def _split_excess_waits(nc):
    """Hoist all-but-one semaphore waits of constrained instructions into
    standalone same-engine EventSemaphore nops placed immediately before them.
    Engine queues are strict FIFO, so semantics are identical."""
    n = 0
    for fn in nc.m.functions:
        for bb in fn.blocks:
            out_insts = []
            for inst in bb.instructions:
                si = inst.sync_info
                if (si is not None and si.on_wait and len(si.on_wait) > 1
                        and type(inst).__name__ in _SPLIT_TYPES):
                    waits = list(si.on_wait)
                    for w in waits[:-1]:
                        out_insts.append(mybir.InstEventSemaphore(
                            name=f"Wsplit-{n}", engine=inst.engine,
                            sync_info=mybir.SyncInfo(on_wait=[w], on_update=[]),
                            ins=[], outs=[]))
                        n += 1
                    si.on_wait = waits[-1:]
                out_insts.append(inst)
            bb.instructions[:] = out_insts


def _smooth_z(z):
    z = np.asarray(z, np.float32)
    g = 1.0
    s = (-2.0 / g ** 3) * z ** 3 + (3.0 / (2.0 * g)) * z + 0.5
    return np.where(z <= -g / 2, 0.0, np.where(z >= g / 2, 1.0, s)).astype(np.float32)


def _prep(inputs):
    """Host-side fold + shard.  Returns (in_maps, const_term)."""
    emb = np.asarray(inputs["emb"], np.float32)
    pairs = np.asarray(inputs["pairs"]).astype(np.int64)
    mains = np.asarray(inputs["mains"]).astype(np.int64)
    pl = np.asarray(inputs["pairs_list"]).astype(np.int64)
    fo = np.asarray(inputs["feature_offsets"]).astype(np.int64)
    pw0 = np.asarray(inputs["pw0"], np.float32)
    pw1 = np.asarray(inputs["pw1"], np.float32)
    pw2 = np.asarray(inputs["pw2"], np.float32)
    pb0 = np.asarray(inputs["pb0"], np.float32)
    pb1 = np.asarray(inputs["pb1"], np.float32)
    pb2 = np.asarray(inputs["pb2"], np.float32)
    mw0 = np.asarray(inputs["mw0"], np.float32)
    mw1 = np.asarray(inputs["mw1"], np.float32)
    mw2 = np.asarray(inputs["mw2"], np.float32)
    mb0 = np.asarray(inputs["mb0"], np.float32)
    mb1 = np.asarray(inputs["mb1"], np.float32)
    mb2 = np.asarray(inputs["mb2"], np.float32)
    zp = _smooth_z(np.asarray(inputs["z_pairs"], np.float32))
    zm = _smooth_z(np.asarray(inputs["z_main"], np.float32))

    # per-feature embedding tables [NF, BINS, E]
    embf = emb[(fo[:, None] + np.arange(BINS)[None, :]).reshape(-1)].reshape(NF, BINS, E)

    # ---- fold layer-0 weights ----
    W0A = np.einsum("pve,peh->pvh", embf[pl[:, 0]], pw0[:, :E, :],
                    optimize=True) + pb0[:, None, :]
    W0B = np.einsum("pve,peh->pvh", embf[pl[:, 1]], pw0[:, E:, :], optimize=True)
    W0stack = np.concatenate([W0A, W0B], axis=1).astype(BF16)   # [P,128,128]
    W0m = (np.einsum("fve,feh->fvh", embf, mw0, optimize=True)
           + mb0[:, None, :]).astype(BF16)                      # [NF,64,128]

    W1p = pw1.astype(BF16)
    W1m = mw1.astype(BF16)
    w2p = (pw2[:, :, 0] * zp[:, None]).astype(BF16)             # [P,128]
    w2m = (mw2[:, :, 0] * zm[:, None]).astype(BF16)             # [NF,128]

    const = float((zp * pb2[:, 0]).sum() + (zm * mb2[:, 0]).sum())

    x = pairs[:, :, 0]  # [B, P]
    y = pairs[:, :, 1]
    bidx = np.arange(B)

    in_maps = []
    for core in range(N_CORES):
        ps = slice(core * PPC, (core + 1) * PPC)
        fs = slice(core * FPC, (core + 1) * FPC)

        M = np.zeros((T, 128, MEGA_F), BF16)
        # pair tasks
        M[:PPC, :, 0:128] = W0stack[ps]
        M[:PPC, :, 128:256] = W1p[ps]
        # one-hot: rows 0..63 <- x, rows 64..127 <- 64+y
        oh = M[:PPC, :, 256:256 + B]  # view [PPC,128,B]
        pidx = np.broadcast_to(np.arange(PPC)[:, None], (PPC, B))
        oh[pidx, x[:, ps].T, np.broadcast_to(bidx[None, :], (PPC, B))] = 1
        oh[pidx, 64 + y[:, ps].T, np.broadcast_to(bidx[None, :], (PPC, B))] = 1
        # main-feature tasks (K padded 64 -> 128 with zeros)
        M[PPC:, 0:BINS, 0:128] = W0m[fs]
        M[PPC:, :, 128:256] = W1m[fs]
        ohm = M[PPC:, :, 256:256 + B]
        fidx = np.broadcast_to(np.arange(FPC)[:, None], (FPC, B))
        ohm[fidx, mains[:, fs].T, np.broadcast_to(bidx[None, :], (FPC, B))] = 1

        megaT = np.ascontiguousarray(M.transpose(1, 0, 2))  # [128, T, MEGA_F]

        pbt = np.zeros((128, T), np.float32)
        pbt[:, :PPC] = pb1[ps].T
        pbt[:, PPC:] = mb1[fs].T
        w2t = np.zeros((128, T), BF16)
        w2t[:, :PPC] = w2p[ps].T
        w2t[:, PPC:] = w2m[fs].T

        in_maps.append({"mega": megaT, "pbt": pbt, "w2t": w2t})

    return in_maps, const


def _enable_tracing():
    """Best-effort: install the axon NTFF profile hook that concourse expects
    (the container's antenv package lacks axon_hooks) and neuter the artifact
    upload.  Only used for test-time profiling; the grading path never traces."""
    import contextlib
    import ctypes
    import types

    from concourse import bass_utils as bu

    bu.upload_artifacts = lambda tmpdir: "local://" + str(tmpdir)
    try:
        import antenv.axon_hooks  # noqa: F401
        return
    except ImportError:
        pass
    so_path = "/opt/axon/libaxon_pjrt.so"
    lib = ctypes.CDLL(so_path)
    if not hasattr(lib, "axon_start_nrt_profile"):
        return
    lib.axon_start_nrt_profile.argtypes = [ctypes.POINTER(ctypes.c_int64),
                                           ctypes.c_size_t]
    lib.axon_start_nrt_profile.restype = ctypes.c_int64
    lib.axon_stop_nrt_profile.argtypes = [ctypes.c_char_p]
    lib.axon_stop_nrt_profile.restype = ctypes.c_int64

    @contextlib.contextmanager
    def _hook(output_dir, device_ids):
        import jax
        jax.devices()
        if device_ids:
            ids = (ctypes.c_int64 * len(device_ids))(*device_ids)
            rc = lib.axon_start_nrt_profile(ids, len(device_ids))
        else:
            rc = lib.axon_start_nrt_profile(None, 0)
        if rc != 0:
            raise RuntimeError(f"axon_start_nrt_profile rc={rc}")
        try:
            yield
        finally:
            n = lib.axon_stop_nrt_profile(str(output_dir).encode())
            print(f"ntff profile: {n} file(s) written to {output_dir}")

    mod = types.ModuleType("antenv.axon_hooks")
    mod.get_axon_ntff_profile_hook = lambda: _hook
    mod.set_axon_ntff_profile_hook = lambda h: None
    import antenv
    antenv.axon_hooks = mod
    sys.modules["antenv.axon_hooks"] = mod


def _run(inputs, trace=False, **kwargs):
    if trace:
        _enable_tracing()
    in_maps, const = _prep(inputs)
    if "nc" not in _cache:
        _cache["nc"] = _build_bass()
    res = run_bass_kernel_spmd(_cache["nc"], in_maps,
                               core_ids=list(range(N_CORES)), trace=trace,
                               **kwargs)
    acc = np.zeros((B,), np.float64)
    for r in res.results:
        acc += r["out"][0].astype(np.float64)
    out = (acc + const).astype(np.float32)[:, None]
    return out, res


def kernel(**inputs):
    out, _ = _run(inputs, trace=False)
    return out


# revision 21
# speedup vs baseline: 1.1128x; 1.1128x over previous
"""DNAMite (dense_mlp) Trainium2 kernel.

Strategy
--------
The model is, per batch row b:
  out[b] = sum_p z_p * MLP_p(concat(emb[x_bp], emb[y_bp]))      (2016 pair MLPs)
         + sum_f z_f * MLP_f(emb[m_bf])                         (64 main MLPs)

Device-side work per (pair|feature) task, with batch B=512 as the matmul
moving dimension:
  1. h0   = W0stack.T @ onehot              (K=128, M=128, N=512, bf16)
  2. a0   = relu(h0)                        (DVE tensor_scalar, PSUM bf16 -> SBUF bf16)
  3. h1   = W1.T @ a0                       (K=128, M=128, N=512, bf16)
  4. a1   = relu(h1 + b1)                   (alternating DVE / ACT)
  5. outP += w2z.T @ a1                     (K=128, M=1, N=512 -> single PSUM fp32 bank,
                                             accumulated across ALL tasks)

Host-side folding (sample-independent weight transforms):
  * The embedding gather + layer-0 matmul are fused:
      W0stack[p] = [emb_i @ pw0[:,:E,:] + pb0 ; emb_j @ pw0[:,E:,:]]  (128x128)
    and the gather itself becomes a {0,1} one-hot rhs built from the integer
    bin indices (exact in bf16).  pb0 folds into the x-half rows since every
    one-hot column has exactly one hit there.
  * z gates fold into the layer-2 weights; z.b2 terms are a scalar constant.

Sharding: 2016 pairs -> 252/core, 64 main features -> 8/core (8 cores).
Each core returns a [1, 512] fp32 partial; the host sums them + constant.
"""

import sys

import numpy as np

for _p in ("/opt/trn_rl_repo",):
    if _p not in sys.path:
        sys.path.insert(0, _p)

import ml_dtypes

import concourse.bass as bass
import concourse.mybir as mybir
import concourse.tile as tile
from concourse.bass_utils import run_bass_kernel_spmd

BF16 = ml_dtypes.bfloat16

N_CORES = 8
B = 512
NF = 64
BINS = 64
E = 32
H = 128
P_TOT = NF * (NF - 1) // 2  # 2016
PPC = P_TOT // N_CORES      # 252 pair tasks per core
FPC = NF // N_CORES         # 8 main-feature tasks per core
T = PPC + FPC               # 260 tasks per core
NJ = T // 2                 # pair-pair (batched) iterations
CH = 13                     # tasks per DMA chunk
assert T % CH == 0 and CH % 2 != 1 or True
PREFETCH = 2                # chunks of DMA lookahead

# free-dim layout of one task row in the mega stream:
#   [0:128)   W0stack lhsT   [K=128 stacked-bin rows, M=128 hidden]
#   [128:256) W1 lhsT        [K=128 hidden, M=128 hidden]
#   [256:768) onehot rhs     [128 stacked-bin rows, B=512]
MEGA_F = 128 + 128 + B

_cache: dict = {}


def _build_bass():
    nc = bass.Bass()
    mega = nc.dram_tensor("mega", [128, T, MEGA_F], mybir.dt.bfloat16,
                          kind="ExternalInput")
    pbt = nc.dram_tensor("pbt", [128, T], mybir.dt.float32, kind="ExternalInput")
    w2t = nc.dram_tensor("w2t", [128, T], mybir.dt.bfloat16, kind="ExternalInput")
    out = nc.dram_tensor("out", [1, B], mybir.dt.float32, kind="ExternalOutput")

    with tile.TileContext(nc) as tc:
        with (
            tc.tile_pool(name="const", bufs=1) as cpool,
            tc.tile_pool(name="mega", bufs=4) as mpool,
            tc.tile_pool(name="act", bufs=10) as apool,
            tc.tile_pool(name="ps", bufs=3, space="PSUM") as pspool,
            tc.tile_pool(name="ps2", bufs=2, space="PSUM") as ps2pool,
            tc.tile_pool(name="po", bufs=1, space="PSUM") as popool,
        ):
            pbt_sb = cpool.tile([128, T], mybir.dt.float32)
            w2t_sb = cpool.tile([128, T], mybir.dt.bfloat16)
            nc.sync.dma_start(pbt_sb[:], pbt[:])
            nc.sync.dma_start(w2t_sb[:], w2t[:])

            # warmup read of pbt_sb on ACT so steady-state activations don't
            # need a DMA wait (walrus allows a single sync wait on ACT ops)
            warm = cpool.tile([128, 1], mybir.dt.float32)
            nc.scalar.activation(warm[:], pbt_sb[:, 0:1],
                                 mybir.ActivationFunctionType.Relu,
                                 bias=0.0, scale=1.0)
            warm2 = cpool.tile([128, 1], mybir.dt.float32)
            nc.vector.tensor_scalar(warm2[:], pbt_sb[:, 0:1], 0.0, None,
                                    mybir.AluOpType.max)

            out_ps = popool.tile([1, B], mybir.dt.float32)

            # Software-pipelined emission over pair-pairs j=(2j, 2j+1).  Per
            # pseudo-iteration the PE stream is mm1(j), mm2(j-2), mm3(j-4) —
            # every matmul's input was produced >=1 iteration earlier, so PE
            # never head-of-line stalls on a DVE/ACT result.  relu0 is batched
            # over both pairs of a pair-pair ([128,1024] spanning 2 PSUM
            # banks) and alternates DVE/ACT; relu1 is per-pair and alternates
            # the other way, balancing the two pointwise engines.
            mega_tiles = {}   # chunk -> tile
            h0_j, a0_j, h1_t, a1_t = {}, {}, {}, {}

            def fetch_chunk(c):
                if c < 0 or c >= T // CH or c in mega_tiles:
                    return
                mt = mpool.tile([128, CH, MEGA_F], mybir.dt.bfloat16)
                nc.sync.dma_start(mt[:], mega[:, c * CH:(c + 1) * CH, :])
                mega_tiles[c] = mt

            def mega_slice(t, lo, hi):
                return mega_tiles[t // CH][:, t % CH, lo:hi]

            for c in range(PREFETCH):
                fetch_chunk(c)

            for u in range(NJ + 4):
                if u < NJ:
                    j = u
                    for t in (2 * j, 2 * j + 1):
                        fetch_chunk(t // CH + PREFETCH)
                    h0 = ps2pool.tile([128, 2 * B], mybir.dt.float32, tag="h0")
                    for i, t in enumerate((2 * j, 2 * j + 1)):
                        nc.tensor.matmul(h0[:, i * B:(i + 1) * B],
                                         mega_slice(t, 0, 128),
                                         mega_slice(t, 256, 256 + B),
                                         start=True, stop=True)
                    h0_j[j] = h0
                if u >= 1 and u - 1 < NJ:
                    j = u - 1
                    a0 = apool.tile([128, 2 * B], mybir.dt.bfloat16, tag="a0")
                    if j % 2 == 0:
                        nc.vector.tensor_scalar(a0[:], h0_j.pop(j)[:], 0.0,
                                                None, mybir.AluOpType.max)
                    else:
                        nc.scalar.activation(a0[:], h0_j.pop(j)[:],
                                             mybir.ActivationFunctionType.Relu,
                                             bias=0.0, scale=1.0)
                    a0_j[j] = a0
                if u >= 2 and u - 2 < NJ:
                    j = u - 2
                    a0 = a0_j.pop(j)
                    for i, t in enumerate((2 * j, 2 * j + 1)):
                        h1 = pspool.tile([128, B], mybir.dt.float32, tag="h1")
                        nc.tensor.matmul(h1[:], mega_slice(t, 128, 256),
                                         a0[:, i * B:(i + 1) * B],
                                         start=True, stop=True)
                        h1_t[t] = h1
                if u >= 3 and u - 3 < NJ:
                    j = u - 3
                    for t in (2 * j, 2 * j + 1):
                        a1 = apool.tile([128, B], mybir.dt.bfloat16, tag="a1")
                        if t % 2 == 0:
                            nc.scalar.activation(
                                a1[:], h1_t.pop(t)[:],
                                mybir.ActivationFunctionType.Relu,
                                bias=pbt_sb[:, t:t + 1], scale=1.0)
                        else:
                            nc.vector.tensor_scalar(a1[:], h1_t.pop(t)[:],
                                                    pbt_sb[:, t:t + 1], 0.0,
                                                    mybir.AluOpType.add,
                                                    mybir.AluOpType.max)
                        a1_t[t] = a1
                if u >= 4:
                    j = u - 4
                    for t in (2 * j, 2 * j + 1):
                        nc.tensor.matmul(out_ps[:], w2t_sb[:, t:t + 1],
                                         a1_t.pop(t)[:],
                                         start=(t == 0), stop=(t == T - 1))

            out_sb = cpool.tile([1, B], mybir.dt.float32)
            nc.vector.tensor_copy(out_sb[:], out_ps[:])
            nc.sync.dma_start(out[:], out_sb[:])

    _split_excess_waits(nc)
    return nc


# Engine-compute opcodes whose ISA structs only carry ONE sync-wait slot in
# this walrus build ("Too many sync wait commands" otherwise).
_SPLIT_TYPES = (
    "InstTensorScalarPtr",
    "InstActivation",
    "InstTensorCopy",
    "InstTensorTensor",
    "InstTensorReduce",
    "InstDMACopy",
    "InstDrain",
    "InstMatmult",
    "InstLdweights",
)


def _split_excess_waits(nc):
    """Hoist all-but-one semaphore waits of constrained instructions into
    standalone same-engine EventSemaphore nops placed immediately before them.
    Engine queues are strict FIFO, so semantics are identical."""
    n = 0
    for fn in nc.m.functions:
        for bb in fn.blocks:
            out_insts = []
            for inst in bb.instructions:
                si = inst.sync_info
                if (si is not None and si.on_wait and len(si.on_wait) > 1
                        and type(inst).__name__ in _SPLIT_TYPES):
                    waits = list(si.on_wait)
                    for w in waits[:-1]:
                        out_insts.append(mybir.InstEventSemaphore(
                            name=f"Wsplit-{n}", engine=inst.engine,
                            sync_info=mybir.SyncInfo(on_wait=[w], on_update=[]),
                            ins=[], outs=[]))
                        n += 1
                    si.on_wait = waits[-1:]
                out_insts.append(inst)
            bb.instructions[:] = out_insts


def _smooth_z(z):
    z = np.asarray(z, np.float32)
    g = 1.0
    s = (-2.0 / g ** 3) * z ** 3 + (3.0 / (2.0 * g)) * z + 0.5
    return np.where(z <= -g / 2, 0.0, np.where(z >= g / 2, 1.0, s)).astype(np.float32)


def _prep(inputs):
    """Host-side fold + shard.  Returns (in_maps, const_term)."""
    emb = np.asarray(inputs["emb"], np.float32)
    pairs = np.asarray(inputs["pairs"]).astype(np.int64)
    mains = np.asarray(inputs["mains"]).astype(np.int64)
    pl = np.asarray(inputs["pairs_list"]).astype(np.int64)
    fo = np.asarray(inputs["feature_offsets"]).astype(np.int64)
    pw0 = np.asarray(inputs["pw0"], np.float32)
    pw1 = np.asarray(inputs["pw1"], np.float32)
    pw2 = np.asarray(inputs["pw2"], np.float32)
    pb0 = np.asarray(inputs["pb0"], np.float32)
    pb1 = np.asarray(inputs["pb1"], np.float32)
    pb2 = np.asarray(inputs["pb2"], np.float32)
    mw0 = np.asarray(inputs["mw0"], np.float32)
    mw1 = np.asarray(inputs["mw1"], np.float32)
    mw2 = np.asarray(inputs["mw2"], np.float32)
    mb0 = np.asarray(inputs["mb0"], np.float32)
    mb1 = np.asarray(inputs["mb1"], np.float32)
    mb2 = np.asarray(inputs["mb2"], np.float32)
    zp = _smooth_z(np.asarray(inputs["z_pairs"], np.float32))
    zm = _smooth_z(np.asarray(inputs["z_main"], np.float32))

    # per-feature embedding tables [NF, BINS, E]
    embf = emb[(fo[:, None] + np.arange(BINS)[None, :]).reshape(-1)].reshape(NF, BINS, E)

    # ---- fold layer-0 weights ----
    W0A = np.einsum("pve,peh->pvh", embf[pl[:, 0]], pw0[:, :E, :],
                    optimize=True) + pb0[:, None, :]
    W0B = np.einsum("pve,peh->pvh", embf[pl[:, 1]], pw0[:, E:, :], optimize=True)
    W0stack = np.concatenate([W0A, W0B], axis=1).astype(BF16)   # [P,128,128]
    W0m = (np.einsum("fve,feh->fvh", embf, mw0, optimize=True)
           + mb0[:, None, :]).astype(BF16)                      # [NF,64,128]

    W1p = pw1.astype(BF16)
    W1m = mw1.astype(BF16)
    w2p = (pw2[:, :, 0] * zp[:, None]).astype(BF16)             # [P,128]
    w2m = (mw2[:, :, 0] * zm[:, None]).astype(BF16)             # [NF,128]

    const = float((zp * pb2[:, 0]).sum() + (zm * mb2[:, 0]).sum())

    x = pairs[:, :, 0]  # [B, P]
    y = pairs[:, :, 1]
    bidx = np.arange(B)

    in_maps = []
    for core in range(N_CORES):
        ps = slice(core * PPC, (core + 1) * PPC)
        fs = slice(core * FPC, (core + 1) * FPC)

        M = np.zeros((T, 128, MEGA_F), BF16)
        # pair tasks
        M[:PPC, :, 0:128] = W0stack[ps]
        M[:PPC, :, 128:256] = W1p[ps]
        # one-hot: rows 0..63 <- x, rows 64..127 <- 64+y
        oh = M[:PPC, :, 256:256 + B]  # view [PPC,128,B]
        pidx = np.broadcast_to(np.arange(PPC)[:, None], (PPC, B))
        oh[pidx, x[:, ps].T, np.broadcast_to(bidx[None, :], (PPC, B))] = 1
        oh[pidx, 64 + y[:, ps].T, np.broadcast_to(bidx[None, :], (PPC, B))] = 1
        # main-feature tasks (K padded 64 -> 128 with zeros)
        M[PPC:, 0:BINS, 0:128] = W0m[fs]
        M[PPC:, :, 128:256] = W1m[fs]
        ohm = M[PPC:, :, 256:256 + B]
        fidx = np.broadcast_to(np.arange(FPC)[:, None], (FPC, B))
        ohm[fidx, mains[:, fs].T, np.broadcast_to(bidx[None, :], (FPC, B))] = 1

        megaT = np.ascontiguousarray(M.transpose(1, 0, 2))  # [128, T, MEGA_F]

        pbt = np.zeros((128, T), np.float32)
        pbt[:, :PPC] = pb1[ps].T
        pbt[:, PPC:] = mb1[fs].T
        w2t = np.zeros((128, T), BF16)
        w2t[:, :PPC] = w2p[ps].T
        w2t[:, PPC:] = w2m[fs].T

        in_maps.append({"mega": megaT, "pbt": pbt, "w2t": w2t})

    return in_maps, const


def _enable_tracing():
    """Best-effort: install the axon NTFF profile hook that concourse expects
    (the container's antenv package lacks axon_hooks) and neuter the artifact
    upload.  Only used for test-time profiling; the grading path never traces."""
    import contextlib
    import ctypes
    import types

    from concourse import bass_utils as bu

    bu.upload_artifacts = lambda tmpdir: "local://" + str(tmpdir)
    try:
        import antenv.axon_hooks  # noqa: F401
        return
    except ImportError:
        pass
    so_path = "/opt/axon/libaxon_pjrt.so"
    lib = ctypes.CDLL(so_path)
    if not hasattr(lib, "axon_start_nrt_profile"):
        return
    lib.axon_start_nrt_profile.argtypes = [ctypes.POINTER(ctypes.c_int64),
                                           ctypes.c_size_t]
    lib.axon_start_nrt_profile.restype = ctypes.c_int64
    lib.axon_stop_nrt_profile.argtypes = [ctypes.c_char_p]
    lib.axon_stop_nrt_profile.restype = ctypes.c_int64

    @contextlib.contextmanager
    def _hook(output_dir, device_ids):
        import jax
        jax.devices()
        if device_ids:
            ids = (ctypes.c_int64 * len(device_ids))(*device_ids)
            rc = lib.axon_start_nrt_profile(ids, len(device_ids))
        else:
            rc = lib.axon_start_nrt_profile(None, 0)
        if rc != 0:
            raise RuntimeError(f"axon_start_nrt_profile rc={rc}")
        try:
            yield
        finally:
            n = lib.axon_stop_nrt_profile(str(output_dir).encode())
            print(f"ntff profile: {n} file(s) written to {output_dir}")

    mod = types.ModuleType("antenv.axon_hooks")
    mod.get_axon_ntff_profile_hook = lambda: _hook
    mod.set_axon_ntff_profile_hook = lambda h: None
    import antenv
    antenv.axon_hooks = mod
    sys.modules["antenv.axon_hooks"] = mod


def _patch_walrus_ldw_opt():
    """Enable walrus's LDWEIGHTS optimization (background weight-buffer use)
    so weight loads overlap matmul streaming; concourse hardcodes it off."""
    from concourse import bass_utils as bu

    if getattr(bu.run_command, "_ldw_patched", False):
        return
    orig = bu.run_command

    def patched(argv, **kw):
        argv = ["--enable-ldw-opt=true" if a == "--enable-ldw-opt=false" else a
                for a in argv]
        return orig(argv, **kw)

    patched._ldw_patched = True
    bu.run_command = patched


def _run(inputs, trace=False, **kwargs):
    # note: walrus "--enable-ldw-opt=true" rejects bass-emitted InstLdweights
    # ("not compatible with LDW optimization"), so it stays off.
    if trace:
        _enable_tracing()
    in_maps, const = _prep(inputs)
    if "nc" not in _cache:
        _cache["nc"] = _build_bass()
    res = run_bass_kernel_spmd(_cache["nc"], in_maps,
                               core_ids=list(range(N_CORES)), trace=trace,
                               **kwargs)
    acc = np.zeros((B,), np.float64)
    for r in res.results:
        acc += r["out"][0].astype(np.float64)
    out = (acc + const).astype(np.float32)[:, None]
    return out, res


def kernel(**inputs):
    out, _ = _run(inputs, trace=False)
    return out
